# revision 12
# baseline (speedup 1.0000x reference)
"""Cross-attention kernel for TRN2, data-parallel over batch (B=8) on 8 cores.

Reference computation per batch element:
    xt  = proj_in(x)              # [L=4096, E=512], 1x1 conv == matmul
    Q   = xt @ W_q.T + b_q
    K   = ctx @ W_k.T + b_k       # ctx: [S=1024, E]
    V   = ctx @ W_v.T + b_v
    att = softmax(Q @ K.T * scale)
    out = proj_out((att @ V).T)   # [C=512, 64, 64]

Host-side algebraic folds (exact up to fp rounding):
  * scale, W_pi, W_q, W_k fold into a single matrix on the Q/K path:
      G = (scale * W_q @ W_pi).T @ W_k ;  logits.T = (G @ ctx).T-contract X
    (the Q'.b_k rank-1 term is constant across keys -> softmax-invariant,
    dropped; a nonzero bias path reappears as per-partition q0 on GC^T X)
  * W_v and W_po fold:  WV = (W_po @ W_v).T ; b_o = b_po + W_po @ b_v
  * softmax normalization is applied at the very end (divide by Z), so the
    attention core is exp -> matmul -> scale-by-1/Z.

On-device, the data-dependent weight products are built ONCE per core
(GC = G.T-contract ctx, VW = ctx.T-contract WV), then every query chunk
needs only the two unavoidable attention GEMMs plus the Z column-sum:
  ST[j,i] = GC.T-contract X ; PT = exp(ST)
  Z[i]    = ones.T @ PT (column sums via PE)
  U[o,i]  = VW.T-contract PT
  y[o,i]  = U * (1/Z broadcast via GpSimd) + b_o

All matmuls run in fp32r (TF32-like: 11-bit mantissa, low 12 bits zero).
DRAM-sourced matmul operands are pre-rounded on the host; device-produced
operands (GC, VW, PT) are rounded by the producing engine via an fp32r
output dtype. PSUM accumulation stays full fp32.
"""

import numpy as np

import concourse.bass as bass
import concourse.mybir as mybir
import concourse.tile as tile
from concourse import bacc
from concourse.bass_utils import run_bass_kernel_spmd

F32 = mybir.dt.float32
F32R = mybir.dt.float32r
EXP = mybir.ActivationFunctionType.Exp

C = 512       # in channels
E = 512       # emb dim
L = 4096      # query length (64*64)
S = 1024      # key length (32*32)
LI = 512      # i-chunk (query) tile size
NCHUNK = L // LI
NCORES = 8

TRACE = False           # test harness can flip this before calling kernel()
LAST_RESULTS = None     # stashed BassKernelResults for the test harness

_PROGRAM_CACHE = {}


def _round_tf32(a: np.ndarray) -> np.ndarray:
    """Round fp32 mantissa to 11 explicit bits (round-to-nearest-even),
    zeroing the low 12 bits — the fp32r operand format."""
    a = np.ascontiguousarray(a, dtype=np.float32)
    b = a.view(np.uint32)
    r = (b + np.uint32(0x7FF) + ((b >> np.uint32(12)) & np.uint32(1))) & np.uint32(
        0xFFFFF000
    )
    return r.view(np.float32)


def _build_program(has_q0: bool, has_bo: bool):
    nc = bacc.Bacc(
        "TRN2",
        target_bir_lowering=False,
        debug=False,
        enable_asserts=False,
        num_devices=NCORES,
    )
    x_d = nc.dram_tensor("x", [C, L], F32R, kind="ExternalInput").ap()
    ctx_d = nc.dram_tensor("ctx", [E, S], F32R, kind="ExternalInput").ap()
    gt_d = nc.dram_tensor("gt", [E, C], F32R, kind="ExternalInput").ap()
    wv_d = nc.dram_tensor("wv", [E, E], F32R, kind="ExternalInput").ap()
    onec_d = nc.dram_tensor("onec", [128, 1], F32R, kind="ExternalInput").ap()
    q0_d = bo_d = None
    if has_q0:
        q0_d = nc.dram_tensor("q0", [128, 8], F32, kind="ExternalInput").ap()
    if has_bo:
        bo_d = nc.dram_tensor("bo", [128, 4], F32, kind="ExternalInput").ap()
    y_d = nc.dram_tensor("y", [C, L], F32, kind="ExternalOutput").ap()

    def load_4stack(pool, dram_ap, width, name):
        """[4*128, width] DRAM -> [128, 4*width] SBUF tile (partition-chunk t
        lands at free offset t*width). One DMA per chunk so consumers of an
        individual chunk can start as soon as that chunk lands (subtile
        deps), and the four transfers spread across DMA queues."""
        t = pool.tile([128, 4 * width], F32R, name=name, tag=name)
        for tt in range(4):
            nc.sync.dma_start(
                t[:, tt * width:(tt + 1) * width],
                dram_ap[tt * 128:(tt + 1) * 128, :],
            )
        return t

    with tile.TileContext(nc) as tc:
        from contextlib import ExitStack

        with ExitStack() as ctx:
            cpool = ctx.enter_context(tc.tile_pool(name="consts", bufs=1))
            ps_s = ctx.enter_context(tc.tile_pool(name="ps_s", bufs=3, space="PSUM"))
            ps_z = ctx.enter_context(tc.tile_pool(name="ps_z", bufs=1, space="PSUM"))
            ps_u = ctx.enter_context(tc.tile_pool(name="ps_u", bufs=4, space="PSUM"))
            xpool = ctx.enter_context(tc.tile_pool(name="xp", bufs=2))
            ppool = ctx.enter_context(tc.tile_pool(name="pp", bufs=2))
            opool = ctx.enter_context(tc.tile_pool(name="op", bufs=2))
            zpool = ctx.enter_context(tc.tile_pool(name="zp", bufs=2))

            # ---- loads in latency-priority order --------------------------
            ones_col = cpool.tile([128, 1], F32R, name="ones_col")
            nc.sync.dma_start(ones_col[:], onec_d[:, :])
            # interleave gt chunks with ctx first-halves so the jh=0 GC
            # groups (which need gt[*] + ctx[*, :LI]) complete after ~2MB of
            # DMA instead of the full 3MB; ctx second-halves follow X0.
            GTS = cpool.tile([128, 4 * C], F32R, name="gstk", tag="gstk")
            CTXT = cpool.tile([128, 4 * S], F32R, name="cstk", tag="cstk")
            for tt in range(4):
                nc.sync.dma_start(
                    GTS[:, tt * C:(tt + 1) * C], gt_d[tt * 128:(tt + 1) * 128, :]
                )
                nc.sync.dma_start(
                    CTXT[:, tt * S:tt * S + LI],
                    ctx_d[tt * 128:(tt + 1) * 128, 0:LI],
                )

            def load_x(ic):
                xt = xpool.tile([128, 4 * LI], F32R, name="xc", tag="x")
                nc.sync.dma_start(
                    xt[:].rearrange("p (t c) -> p t c", c=LI),
                    x_d[:, bass.ts(ic, LI)].rearrange("(t p) c -> p t c", p=128),
                )
                return xt

            X0 = load_x(0)                                        # prefetch chunk 0
            for tt in range(4):
                nc.sync.dma_start(
                    CTXT[:, tt * S + LI:(tt + 1) * S],
                    ctx_d[tt * 128:(tt + 1) * 128, LI:S],
                )
            WVT = load_4stack(cpool, wv_d[:, :], E, "wstk")       # [128, 4*E]
            q0_s = bo_s = None
            if has_q0:
                q0_s = cpool.tile([128, 8], F32, name="q0s")
                nc.sync.dma_start(q0_s[:], q0_d[:, :])
            if has_bo:
                bo_s = cpool.tile([128, 4], F32, name="bos")
                nc.sync.dma_start(bo_s[:], bo_d[:, :])

            def ctx_blk(et, jt):            # CTX [e-chunk et, j-tile jt]
                return CTXT[:, et * S + jt * 128: et * S + (jt + 1) * 128]

            # ---- PE warm-up -----------------------------------------------
            # The PE's HAM clock gate keeps it at 1.2 GHz until ~3.4us of
            # sustained activity. Real work can't start until gt+ctx land
            # (~15us), so spin throwaway matmuls on the first-landed gt
            # chunk during the DMA wait; GC then starts at full 2.4 GHz.
            for w in range(14):
                wps = ps_z.tile([128, LI], F32, name="wps", tag="z")
                nc.tensor.matmul(
                    wps[:], GTS[:, 0:128], GTS[:, 0:LI], start=True, stop=True
                )

            # ---- GC[c, j] = sum_e G[c, e] ctx[e, j]  (Q/K path, once) ----
            # jh-outer: the four jh=0 groups need only the ctx first-halves,
            # which are the first DMAs to land.
            GC = [
                cpool.tile([128, S], F32R, name=f"gc{ct}", tag=f"gc{ct}")
                for ct in range(4)
            ]
            for jh in range(2):
                for ct in range(4):
                    gps = ps_s.tile([128, LI], F32, name="gps", tag="s")
                    for et in range(4):
                        nc.tensor.matmul(
                            gps[:],
                            GTS[:, et * C + ct * 128: et * C + (ct + 1) * 128],
                            CTXT[:, et * S + jh * LI: et * S + (jh + 1) * LI],
                            start=(et == 0),
                            stop=(et == 3),
                        )
                    nc.vector.tensor_copy(GC[ct][:, jh * LI:(jh + 1) * LI], gps[:])

            X = X0
            for ic in range(NCHUNK):
                isl = bass.ts(ic, LI)
                Xc = X
                if ic + 1 < NCHUNK:
                    X = load_x(ic + 1)      # prefetch next chunk
                # ST[j, i] = GC.T-contract X (+ q0[j]) ; PT = exp(ST).
                # The Z partial-sum tree (pairwise adds on the DVE) is
                # interleaved into the S-loop so each add issues as soon as
                # its exp operands exist; the final 128-partition fold is a
                # single ones-matmul placed after the first U-group so the
                # in-order PE stream never waits on the DVE.
                PT = []
                tpart = {}
                for jt in range(8):
                    sps = ps_s.tile([128, LI], F32, name="sps", tag="s")
                    for ct in range(4):
                        nc.tensor.matmul(
                            sps[:],
                            GC[ct][:, jt * 128:(jt + 1) * 128],
                            Xc[:, bass.ts(ct, LI)],
                            start=(ct == 0),
                            stop=(ct == 3),
                        )
                    p = ppool.tile([128, LI], F32R, name="pt", tag=f"p{jt}")
                    if has_q0:
                        nc.scalar.activation(
                            p[:], sps[:], EXP, bias=q0_s[:, jt:jt + 1]
                        )
                    else:
                        nc.scalar.activation(p[:], sps[:], EXP)
                    PT.append(p)
                    if jt in (1, 3, 5, 7):
                        t = zpool.tile([128, LI], F32, name="tp", tag=f"t{jt // 2}")
                        nc.vector.tensor_add(
                            t[:],
                            PT[jt - 1][:].bitcast(F32),
                            PT[jt][:].bitcast(F32),
                        )
                        tpart[jt // 2] = t
                    if jt == 3:
                        ta = zpool.tile([128, LI], F32, name="ta", tag="ta")
                        nc.vector.tensor_add(ta[:], tpart[0][:], tpart[1][:])
                    if jt == 7:
                        tb = zpool.tile([128, LI], F32, name="tb", tag="tb")
                        nc.vector.tensor_add(tb[:], tpart[2][:], tpart[3][:])
                        zt = zpool.tile([128, LI], F32R, name="zt", tag="zt")
                        nc.vector.tensor_add(zt[:], ta[:], tb[:])
                if ic == 0:
                    # VW[j, o] = sum_e ctx[e, j] WV[e, o] (V/out path, once).
                    # Emitted between chunk 0's S and U stages so it hides in
                    # the exp/Z latency instead of delaying the first chunk.
                    VW = []
                    for jt in range(8):
                        vps = ps_s.tile([128, E], F32, name="vps", tag="s")
                        for et in range(4):
                            nc.tensor.matmul(
                                vps[:],
                                ctx_blk(et, jt),
                                WVT[:, bass.ts(et, E)],
                                start=(et == 0),
                                stop=(et == 3),
                            )
                        vw = cpool.tile(
                            [128, E], F32R, name=f"vwt{jt}", tag=f"vwt{jt}"
                        )
                        nc.vector.tensor_copy(vw[:], vps[:])
                        VW.append(vw)
                # U[o, i] = VW.T-contract PT ; y = U * invZ (+ b_o)
                for ot in range(4):
                    ups = ps_u.tile([128, LI], F32, name="ups", tag="u")
                    for jt in range(8):
                        nc.tensor.matmul(
                            ups[:],
                            VW[jt][:, ot * 128:(ot + 1) * 128],
                            PT[jt][:],
                            start=(jt == 0),
                            stop=(jt == 7),
                        )
                    if ot == 0:
                        zps = ps_z.tile([1, LI], F32, name="zps", tag="z")
                        nc.tensor.matmul(
                            zps[:], ones_col[:], zt[:], start=True, stop=True
                        )
                        invz = zpool.tile([1, LI], F32, name="invz", tag="invz")
                        nc.vector.reciprocal(invz[:], zps[:])
                        invz_rep = zpool.tile(
                            [128, LI], F32, name="invz_rep", tag="invzrep"
                        )
                        nc.gpsimd.partition_broadcast(invz_rep[:], invz[:])
                    o = opool.tile([128, LI], F32, name="ot", tag=f"o{ot}")
                    nc.vector.tensor_mul(o[:], ups[:], invz_rep[:])
                    if has_bo:
                        nc.vector.tensor_scalar_add(o[:], o[:], bo_s[:, ot:ot + 1])
                    nc.sync.dma_start(y_d[ot * 128:(ot + 1) * 128, isl], o[:])

    nc.compile()
    return nc


def kernel(**inputs) -> np.ndarray:
    global LAST_RESULTS
    x = np.asarray(inputs["x"], dtype=np.float32)
    context = np.asarray(inputs["context"], dtype=np.float32)
    W_pi = np.asarray(inputs["W_pi"], dtype=np.float64)
    b_pi = np.asarray(inputs["b_pi"], dtype=np.float64)
    W_q = np.asarray(inputs["W_q"], dtype=np.float64)
    b_q = np.asarray(inputs["b_q"], dtype=np.float64)
    W_k = np.asarray(inputs["W_k"], dtype=np.float64)
    W_v = np.asarray(inputs["W_v"], dtype=np.float64)
    b_v = np.asarray(inputs["b_v"], dtype=np.float64)
    W_po = np.asarray(inputs["W_po"], dtype=np.float64)
    b_po = np.asarray(inputs["b_po"], dtype=np.float64)

    scale = float(E) ** -0.5
    Wqpi = scale * (W_q @ W_pi)                            # [dq, c]
    G = (Wqpi.T @ W_k)                                     # [c, e]
    GT = _round_tf32(np.ascontiguousarray(G.T).astype(np.float32))  # [e, c]
    b_row = scale * (W_q @ b_pi + b_q)
    # per-KEY bias on the logits: q0[j] = (W_k.T b_row) . ctx[:, j] is handled
    # as an activation bias per j-partition, computed from ctx on the host
    # would be data-work; instead fold the e-space bias through the device GC
    # path is impossible (it multiplies ctx), so compute the per-j bias here
    # only when biases are actually nonzero (they are all zero in this
    # problem's inputs).
    q0_e = (W_k.T @ b_row).astype(np.float64)              # [e]
    WV = _round_tf32((W_po @ W_v).T.astype(np.float32))    # [e, o]
    b_o = (b_po + W_po @ b_v).astype(np.float32)           # [o]

    has_q0 = bool(np.any(q0_e))
    has_bo = bool(np.any(b_o))
    key = (has_q0, has_bo)
    if key not in _PROGRAM_CACHE:
        _PROGRAM_CACHE[key] = _build_program(has_q0, has_bo)
    nc = _PROGRAM_CACHE[key]

    ones_c = np.ones((128, 1), dtype=np.float32)
    in_maps = []
    for c in range(NCORES):
        ctx_mat = context[c].reshape(E, S)
        m = {
            "x": _round_tf32(x[c].reshape(C, L)),
            "ctx": _round_tf32(ctx_mat),
            "gt": GT,
            "wv": WV,
            "onec": ones_c,
        }
        if has_q0:
            # logits bias per key j: q0_e . ctx[:, j]  -> [S] -> [128, 8]
            q0j = (q0_e @ ctx_mat.astype(np.float64)).astype(np.float32)
            m["q0"] = np.ascontiguousarray(q0j.reshape(8, 128).T)
        if has_bo:
            m["bo"] = np.ascontiguousarray(b_o.reshape(4, 128).T)
        in_maps.append(m)

    res = run_bass_kernel_spmd(nc, in_maps, core_ids=list(range(NCORES)), trace=TRACE)
    LAST_RESULTS = res
    y = np.stack([res.results[c]["y"] for c in range(NCORES)], axis=0)
    return np.ascontiguousarray(y.reshape(NCORES, C, 64, 64).astype(np.float32))


# revision 13
# speedup vs baseline: 1.0054x; 1.0054x over previous
"""Cross-attention kernel for TRN2, data-parallel over batch (B=8) on 8 cores.

Reference computation per batch element:
    xt  = proj_in(x)              # [L=4096, E=512], 1x1 conv == matmul
    Q   = xt @ W_q.T + b_q
    K   = ctx @ W_k.T + b_k       # ctx: [S=1024, E]
    V   = ctx @ W_v.T + b_v
    att = softmax(Q @ K.T * scale)
    out = proj_out((att @ V).T)   # [C=512, 64, 64]

Host-side algebraic folds (exact up to fp rounding):
  * scale, W_pi, W_q, W_k fold into a single matrix on the Q/K path:
      G = (scale * W_q @ W_pi).T @ W_k ;  logits.T = (G @ ctx).T-contract X
    (the Q'.b_k rank-1 term is constant across keys -> softmax-invariant,
    dropped; a nonzero bias path reappears as per-partition q0 on GC^T X)
  * W_v and W_po fold:  WV = (W_po @ W_v).T ; b_o = b_po + W_po @ b_v
  * softmax normalization is applied at the very end (divide by Z), so the
    attention core is exp -> matmul -> scale-by-1/Z.

On-device, the data-dependent weight products are built ONCE per core
(GC = G.T-contract ctx, VW = ctx.T-contract WV), then every query chunk
needs only the two unavoidable attention GEMMs plus the Z column-sum:
  ST[j,i] = GC.T-contract X ; PT = exp(ST)
  Z[i]    = ones.T @ PT (column sums via PE)
  U[o,i]  = VW.T-contract PT
  y[o,i]  = U * (1/Z broadcast via GpSimd) + b_o

All matmuls run in fp32r (TF32-like: 11-bit mantissa, low 12 bits zero).
DRAM-sourced matmul operands are pre-rounded on the host; device-produced
operands (GC, VW, PT) are rounded by the producing engine via an fp32r
output dtype. PSUM accumulation stays full fp32.
"""

import numpy as np

import concourse.bass as bass
import concourse.mybir as mybir
import concourse.tile as tile
from concourse import bacc
from concourse.bass_utils import run_bass_kernel_spmd

F32 = mybir.dt.float32
F32R = mybir.dt.float32r
EXP = mybir.ActivationFunctionType.Exp

C = 512       # in channels
E = 512       # emb dim
L = 4096      # query length (64*64)
S = 1024      # key length (32*32)
LI = 512      # i-chunk (query) tile size
NCHUNK = L // LI
NCORES = 8

TRACE = False           # test harness can flip this before calling kernel()
LAST_RESULTS = None     # stashed BassKernelResults for the test harness

_PROGRAM_CACHE = {}


def _round_tf32(a: np.ndarray) -> np.ndarray:
    """Round fp32 mantissa to 11 explicit bits (round-to-nearest-even),
    zeroing the low 12 bits — the fp32r operand format."""
    a = np.ascontiguousarray(a, dtype=np.float32)
    b = a.view(np.uint32)
    r = (b + np.uint32(0x7FF) + ((b >> np.uint32(12)) & np.uint32(1))) & np.uint32(
        0xFFFFF000
    )
    return r.view(np.float32)


def _build_program(has_q0: bool, has_bo: bool):
    nc = bacc.Bacc(
        "TRN2",
        target_bir_lowering=False,
        debug=False,
        enable_asserts=False,
        num_devices=NCORES,
    )
    x_d = nc.dram_tensor("x", [C, L], F32R, kind="ExternalInput").ap()
    ctx_d = nc.dram_tensor("ctx", [E, S], F32R, kind="ExternalInput").ap()
    gt_d = nc.dram_tensor("gt", [E, C], F32R, kind="ExternalInput").ap()
    wv_d = nc.dram_tensor("wv", [E, E], F32R, kind="ExternalInput").ap()
    onec_d = nc.dram_tensor("onec", [128, 1], F32R, kind="ExternalInput").ap()
    q0_d = bo_d = None
    if has_q0:
        q0_d = nc.dram_tensor("q0", [128, 8], F32, kind="ExternalInput").ap()
    if has_bo:
        bo_d = nc.dram_tensor("bo", [128, 4], F32, kind="ExternalInput").ap()
    y_d = nc.dram_tensor("y", [C, L], F32, kind="ExternalOutput").ap()

    def load_4stack(pool, dram_ap, width, name):
        """[4*128, width] DRAM -> [128, 4*width] SBUF tile (partition-chunk t
        lands at free offset t*width). One DMA per chunk so consumers of an
        individual chunk can start as soon as that chunk lands (subtile
        deps), and the four transfers spread across DMA queues."""
        t = pool.tile([128, 4 * width], F32R, name=name, tag=name)
        for tt in range(4):
            nc.sync.dma_start(
                t[:, tt * width:(tt + 1) * width],
                dram_ap[tt * 128:(tt + 1) * 128, :],
            )
        return t

    with tile.TileContext(nc) as tc:
        from contextlib import ExitStack

        with ExitStack() as ctx:
            cpool = ctx.enter_context(tc.tile_pool(name="consts", bufs=1))
            ps_s = ctx.enter_context(tc.tile_pool(name="ps_s", bufs=3, space="PSUM"))
            ps_z = ctx.enter_context(tc.tile_pool(name="ps_z", bufs=1, space="PSUM"))
            ps_u = ctx.enter_context(tc.tile_pool(name="ps_u", bufs=4, space="PSUM"))
            xpool = ctx.enter_context(tc.tile_pool(name="xp", bufs=2))
            ppool = ctx.enter_context(tc.tile_pool(name="pp", bufs=2))
            opool = ctx.enter_context(tc.tile_pool(name="op", bufs=2))
            zpool = ctx.enter_context(tc.tile_pool(name="zp", bufs=2))

            # ---- loads in latency-priority order --------------------------
            ones_col = cpool.tile([128, 1], F32R, name="ones_col")
            nc.sync.dma_start(ones_col[:], onec_d[:, :])
            # interleave gt chunks with ctx first-halves so the jh=0 GC
            # groups (which need gt[*] + ctx[*, :LI]) complete after ~2MB of
            # DMA instead of the full 3MB; ctx second-halves follow X0.
            GTS = cpool.tile([128, 4 * C], F32R, name="gstk", tag="gstk")
            CTXT = cpool.tile([128, 4 * S], F32R, name="cstk", tag="cstk")
            for tt in range(4):
                nc.sync.dma_start(
                    GTS[:, tt * C:(tt + 1) * C], gt_d[tt * 128:(tt + 1) * 128, :]
                )
                nc.sync.dma_start(
                    CTXT[:, tt * S:tt * S + LI],
                    ctx_d[tt * 128:(tt + 1) * 128, 0:LI],
                )

            def load_x(ic):
                xt = xpool.tile([128, 4 * LI], F32R, name="xc", tag="x")
                nc.sync.dma_start(
                    xt[:].rearrange("p (t c) -> p t c", c=LI),
                    x_d[:, bass.ts(ic, LI)].rearrange("(t p) c -> p t c", p=128),
                )
                return xt

            X0 = load_x(0)                                        # prefetch chunk 0
            for tt in range(4):
                nc.sync.dma_start(
                    CTXT[:, tt * S + LI:(tt + 1) * S],
                    ctx_d[tt * 128:(tt + 1) * 128, LI:S],
                )
            WVT = load_4stack(cpool, wv_d[:, :], E, "wstk")       # [128, 4*E]
            q0_s = bo_s = None
            if has_q0:
                q0_s = cpool.tile([128, 8], F32, name="q0s")
                nc.sync.dma_start(q0_s[:], q0_d[:, :])
            if has_bo:
                bo_s = cpool.tile([128, 4], F32, name="bos")
                nc.sync.dma_start(bo_s[:], bo_d[:, :])

            def ctx_blk(et, jt):            # CTX [e-chunk et, j-tile jt]
                return CTXT[:, et * S + jt * 128: et * S + (jt + 1) * 128]

            # ---- GC[c, j] = sum_e G[c, e] ctx[e, j]  (Q/K path, once) ----
            # jh-outer: the four jh=0 groups need only the ctx first-halves,
            # which are the first DMAs to land.
            GC = [
                cpool.tile([128, S], F32R, name=f"gc{ct}", tag=f"gc{ct}")
                for ct in range(4)
            ]
            for jh in range(2):
                for ct in range(4):
                    gps = ps_s.tile([128, LI], F32, name="gps", tag="s")
                    for et in range(4):
                        nc.tensor.matmul(
                            gps[:],
                            GTS[:, et * C + ct * 128: et * C + (ct + 1) * 128],
                            CTXT[:, et * S + jh * LI: et * S + (jh + 1) * LI],
                            start=(et == 0),
                            stop=(et == 3),
                        )
                    nc.vector.tensor_copy(GC[ct][:, jh * LI:(jh + 1) * LI], gps[:])

            X = X0
            for ic in range(NCHUNK):
                isl = bass.ts(ic, LI)
                Xc = X
                if ic + 1 < NCHUNK:
                    X = load_x(ic + 1)      # prefetch next chunk
                # ST[j, i] = GC.T-contract X (+ q0[j]) ; PT = exp(ST).
                # The Z partial-sum tree (pairwise adds on the DVE) is
                # interleaved into the S-loop so each add issues as soon as
                # its exp operands exist; the final 128-partition fold is a
                # single ones-matmul placed after the first U-group so the
                # in-order PE stream never waits on the DVE.
                PT = []
                tpart = {}
                for jt in range(8):
                    sps = ps_s.tile([128, LI], F32, name="sps", tag="s")
                    for ct in range(4):
                        nc.tensor.matmul(
                            sps[:],
                            GC[ct][:, jt * 128:(jt + 1) * 128],
                            Xc[:, bass.ts(ct, LI)],
                            start=(ct == 0),
                            stop=(ct == 3),
                        )
                    p = ppool.tile([128, LI], F32R, name="pt", tag=f"p{jt}")
                    if has_q0:
                        nc.scalar.activation(
                            p[:], sps[:], EXP, bias=q0_s[:, jt:jt + 1]
                        )
                    else:
                        nc.scalar.activation(p[:], sps[:], EXP)
                    PT.append(p)
                    if jt in (1, 3, 5, 7):
                        t = zpool.tile([128, LI], F32, name="tp", tag=f"t{jt // 2}")
                        nc.vector.tensor_add(
                            t[:],
                            PT[jt - 1][:].bitcast(F32),
                            PT[jt][:].bitcast(F32),
                        )
                        tpart[jt // 2] = t
                    if jt == 3:
                        ta = zpool.tile([128, LI], F32, name="ta", tag="ta")
                        nc.vector.tensor_add(ta[:], tpart[0][:], tpart[1][:])
                    if jt == 7:
                        tb = zpool.tile([128, LI], F32, name="tb", tag="tb")
                        nc.vector.tensor_add(tb[:], tpart[2][:], tpart[3][:])
                        zt = zpool.tile([128, LI], F32R, name="zt", tag="zt")
                        nc.vector.tensor_add(zt[:], ta[:], tb[:])
                if ic == 0:
                    # VW[j, o] = sum_e ctx[e, j] WV[e, o] (V/out path, once).
                    # Emitted between chunk 0's S and U stages so it hides in
                    # the exp/Z latency instead of delaying the first chunk.
                    VW = []
                    for jt in range(8):
                        vps = ps_s.tile([128, E], F32, name="vps", tag="s")
                        for et in range(4):
                            nc.tensor.matmul(
                                vps[:],
                                ctx_blk(et, jt),
                                WVT[:, bass.ts(et, E)],
                                start=(et == 0),
                                stop=(et == 3),
                            )
                        vw = cpool.tile(
                            [128, E], F32R, name=f"vwt{jt}", tag=f"vwt{jt}"
                        )
                        nc.vector.tensor_copy(vw[:], vps[:])
                        VW.append(vw)
                # U[o, i] = VW.T-contract PT ; y = U * invZ (+ b_o)
                for ot in range(4):
                    ups = ps_u.tile([128, LI], F32, name="ups", tag="u")
                    for jt in range(8):
                        nc.tensor.matmul(
                            ups[:],
                            VW[jt][:, ot * 128:(ot + 1) * 128],
                            PT[jt][:],
                            start=(jt == 0),
                            stop=(jt == 7),
                        )
                    if ot == 0:
                        zps = ps_z.tile([1, LI], F32, name="zps", tag="z")
                        nc.tensor.matmul(
                            zps[:], ones_col[:], zt[:], start=True, stop=True
                        )
                        invz = zpool.tile([1, LI], F32, name="invz", tag="invz")
                        nc.vector.reciprocal(invz[:], zps[:])
                        invz_rep = zpool.tile(
                            [128, LI], F32, name="invz_rep", tag="invzrep"
                        )
                        nc.gpsimd.partition_broadcast(invz_rep[:], invz[:])
                    o = opool.tile([128, LI], F32, name="ot", tag=f"o{ot}")
                    nc.vector.tensor_mul(o[:], ups[:], invz_rep[:])
                    if has_bo:
                        nc.vector.tensor_scalar_add(o[:], o[:], bo_s[:, ot:ot + 1])
                    nc.sync.dma_start(y_d[ot * 128:(ot + 1) * 128, isl], o[:])

    nc.compile()
    return nc


def kernel(**inputs) -> np.ndarray:
    global LAST_RESULTS
    x = np.asarray(inputs["x"], dtype=np.float32)
    context = np.asarray(inputs["context"], dtype=np.float32)
    W_pi = np.asarray(inputs["W_pi"], dtype=np.float64)
    b_pi = np.asarray(inputs["b_pi"], dtype=np.float64)
    W_q = np.asarray(inputs["W_q"], dtype=np.float64)
    b_q = np.asarray(inputs["b_q"], dtype=np.float64)
    W_k = np.asarray(inputs["W_k"], dtype=np.float64)
    W_v = np.asarray(inputs["W_v"], dtype=np.float64)
    b_v = np.asarray(inputs["b_v"], dtype=np.float64)
    W_po = np.asarray(inputs["W_po"], dtype=np.float64)
    b_po = np.asarray(inputs["b_po"], dtype=np.float64)

    scale = float(E) ** -0.5
    Wqpi = scale * (W_q @ W_pi)                            # [dq, c]
    G = (Wqpi.T @ W_k)                                     # [c, e]
    GT = _round_tf32(np.ascontiguousarray(G.T).astype(np.float32))  # [e, c]
    b_row = scale * (W_q @ b_pi + b_q)
    # per-KEY bias on the logits: q0[j] = (W_k.T b_row) . ctx[:, j] is handled
    # as an activation bias per j-partition, computed from ctx on the host
    # would be data-work; instead fold the e-space bias through the device GC
    # path is impossible (it multiplies ctx), so compute the per-j bias here
    # only when biases are actually nonzero (they are all zero in this
    # problem's inputs).
    q0_e = (W_k.T @ b_row).astype(np.float64)              # [e]
    WV = _round_tf32((W_po @ W_v).T.astype(np.float32))    # [e, o]
    b_o = (b_po + W_po @ b_v).astype(np.float32)           # [o]

    has_q0 = bool(np.any(q0_e))
    has_bo = bool(np.any(b_o))
    key = (has_q0, has_bo)
    if key not in _PROGRAM_CACHE:
        _PROGRAM_CACHE[key] = _build_program(has_q0, has_bo)
    nc = _PROGRAM_CACHE[key]

    ones_c = np.ones((128, 1), dtype=np.float32)
    in_maps = []
    for c in range(NCORES):
        ctx_mat = context[c].reshape(E, S)
        m = {
            "x": _round_tf32(x[c].reshape(C, L)),
            "ctx": _round_tf32(ctx_mat),
            "gt": GT,
            "wv": WV,
            "onec": ones_c,
        }
        if has_q0:
            # logits bias per key j: q0_e . ctx[:, j]  -> [S] -> [128, 8]
            q0j = (q0_e @ ctx_mat.astype(np.float64)).astype(np.float32)
            m["q0"] = np.ascontiguousarray(q0j.reshape(8, 128).T)
        if has_bo:
            m["bo"] = np.ascontiguousarray(b_o.reshape(4, 128).T)
        in_maps.append(m)

    res = run_bass_kernel_spmd(nc, in_maps, core_ids=list(range(NCORES)), trace=TRACE)
    LAST_RESULTS = res
    y = np.stack([res.results[c]["y"] for c in range(NCORES)], axis=0)
    return np.ascontiguousarray(y.reshape(NCORES, C, 64, 64).astype(np.float32))


# revision 14
# speedup vs baseline: 1.0161x; 1.0106x over previous
"""Cross-attention kernel for TRN2, data-parallel over batch (B=8) on 8 cores.

Reference computation per batch element:
    xt  = proj_in(x)              # [L=4096, E=512], 1x1 conv == matmul
    Q   = xt @ W_q.T + b_q
    K   = ctx @ W_k.T + b_k       # ctx: [S=1024, E]
    V   = ctx @ W_v.T + b_v
    att = softmax(Q @ K.T * scale)
    out = proj_out((att @ V).T)   # [C=512, 64, 64]

Host-side algebraic folds (exact up to fp rounding):
  * scale, W_pi, W_q, W_k fold into a single matrix on the Q/K path:
      G = (scale * W_q @ W_pi).T @ W_k ;  logits.T = (G @ ctx).T-contract X
    (the Q'.b_k rank-1 term is constant across keys -> softmax-invariant,
    dropped; a nonzero bias path reappears as per-partition q0 on GC^T X)
  * W_v and W_po fold:  WV = (W_po @ W_v).T ; b_o = b_po + W_po @ b_v
  * softmax normalization is applied at the very end (divide by Z), so the
    attention core is exp -> matmul -> scale-by-1/Z.

On-device, the data-dependent weight products are built ONCE per core
(GC = G.T-contract ctx, VW = ctx.T-contract WV), then every query chunk
needs only the two unavoidable attention GEMMs plus the Z column-sum:
  ST[j,i] = GC.T-contract X ; PT = exp(ST)
  Z[i]    = ones.T @ PT (column sums via PE)
  U[o,i]  = VW.T-contract PT
  y[o,i]  = U * (1/Z broadcast via GpSimd) + b_o

All matmuls run in fp32r (TF32-like: 11-bit mantissa, low 12 bits zero).
DRAM-sourced matmul operands are pre-rounded on the host; device-produced
operands (GC, VW, PT) are rounded by the producing engine via an fp32r
output dtype. PSUM accumulation stays full fp32.
"""

import numpy as np

import concourse.bass as bass
import concourse.mybir as mybir
import concourse.tile as tile
from concourse import bacc
from concourse.bass_utils import run_bass_kernel_spmd

F32 = mybir.dt.float32
F32R = mybir.dt.float32r
EXP = mybir.ActivationFunctionType.Exp

C = 512       # in channels
E = 512       # emb dim
L = 4096      # query length (64*64)
S = 1024      # key length (32*32)
LI = 512      # i-chunk (query) tile size
NCHUNK = L // LI
NCORES = 8

TRACE = False           # test harness can flip this before calling kernel()
LAST_RESULTS = None     # stashed BassKernelResults for the test harness

_PROGRAM_CACHE = {}


def _round_tf32(a: np.ndarray) -> np.ndarray:
    """Round fp32 mantissa to 11 explicit bits (round-to-nearest-even),
    zeroing the low 12 bits — the fp32r operand format."""
    a = np.ascontiguousarray(a, dtype=np.float32)
    b = a.view(np.uint32)
    r = (b + np.uint32(0x7FF) + ((b >> np.uint32(12)) & np.uint32(1))) & np.uint32(
        0xFFFFF000
    )
    return r.view(np.float32)


def _build_program(has_q0: bool, has_bo: bool):
    nc = bacc.Bacc(
        "TRN2",
        target_bir_lowering=False,
        debug=False,
        enable_asserts=False,
        num_devices=NCORES,
    )
    x_d = nc.dram_tensor("x", [C, L], F32R, kind="ExternalInput").ap()
    ctx_d = nc.dram_tensor("ctx", [E, S], F32R, kind="ExternalInput").ap()
    gt_d = nc.dram_tensor("gt", [E, C], F32R, kind="ExternalInput").ap()
    wv_d = nc.dram_tensor("wv", [E, E], F32R, kind="ExternalInput").ap()
    onec_d = nc.dram_tensor("onec", [128, 1], F32R, kind="ExternalInput").ap()
    q0_d = bo_d = None
    if has_q0:
        q0_d = nc.dram_tensor("q0", [128, 8], F32, kind="ExternalInput").ap()
    if has_bo:
        bo_d = nc.dram_tensor("bo", [128, 4], F32, kind="ExternalInput").ap()
    y_d = nc.dram_tensor("y", [C, L], F32, kind="ExternalOutput").ap()

    def load_4stack(pool, dram_ap, width, name):
        """[4*128, width] DRAM -> [128, 4*width] SBUF tile (partition-chunk t
        lands at free offset t*width). One DMA per chunk so consumers of an
        individual chunk can start as soon as that chunk lands (subtile
        deps), and the four transfers spread across DMA queues."""
        t = pool.tile([128, 4 * width], F32R, name=name, tag=name)
        for tt in range(4):
            nc.sync.dma_start(
                t[:, tt * width:(tt + 1) * width],
                dram_ap[tt * 128:(tt + 1) * 128, :],
            )
        return t

    with tile.TileContext(nc) as tc:
        from contextlib import ExitStack

        with ExitStack() as ctx:
            cpool = ctx.enter_context(tc.tile_pool(name="consts", bufs=1))
            ps_s = ctx.enter_context(tc.tile_pool(name="ps_s", bufs=3, space="PSUM"))
            ps_z = ctx.enter_context(tc.tile_pool(name="ps_z", bufs=1, space="PSUM"))
            ps_u = ctx.enter_context(tc.tile_pool(name="ps_u", bufs=4, space="PSUM"))
            xpool = ctx.enter_context(tc.tile_pool(name="xp", bufs=2))
            ppool = ctx.enter_context(tc.tile_pool(name="pp", bufs=2))
            opool = ctx.enter_context(tc.tile_pool(name="op", bufs=2))
            zpool = ctx.enter_context(tc.tile_pool(name="zp", bufs=2))

            # ---- loads in latency-priority order --------------------------
            ones_col = cpool.tile([128, 1], F32R, name="ones_col")
            nc.sync.dma_start(ones_col[:], onec_d[:, :])
            # interleave gt chunks with ctx first-halves so the jh=0 GC
            # groups (which need gt[*] + ctx[*, :LI]) complete after ~2MB of
            # DMA instead of the full 3MB; ctx second-halves follow X0.
            GTS = cpool.tile([128, 4 * C], F32R, name="gstk", tag="gstk")
            CTXT = cpool.tile([128, 4 * S], F32R, name="cstk", tag="cstk")
            for tt in range(4):
                nc.sync.dma_start(
                    GTS[:, tt * C:(tt + 1) * C], gt_d[tt * 128:(tt + 1) * 128, :]
                )
                nc.sync.dma_start(
                    CTXT[:, tt * S:tt * S + LI],
                    ctx_d[tt * 128:(tt + 1) * 128, 0:LI],
                )

            def load_x(ic):
                xt = xpool.tile([128, 4 * LI], F32R, name="xc", tag="x")
                nc.sync.dma_start(
                    xt[:].rearrange("p (t c) -> p t c", c=LI),
                    x_d[:, bass.ts(ic, LI)].rearrange("(t p) c -> p t c", p=128),
                )
                return xt

            X0 = load_x(0)                                        # prefetch chunk 0
            for tt in range(4):
                nc.sync.dma_start(
                    CTXT[:, tt * S + LI:(tt + 1) * S],
                    ctx_d[tt * 128:(tt + 1) * 128, LI:S],
                )
            WVT = load_4stack(cpool, wv_d[:, :], E, "wstk")       # [128, 4*E]
            q0_s = bo_s = None
            if has_q0:
                q0_s = cpool.tile([128, 8], F32, name="q0s")
                nc.sync.dma_start(q0_s[:], q0_d[:, :])
            if has_bo:
                bo_s = cpool.tile([128, 4], F32, name="bos")
                nc.sync.dma_start(bo_s[:], bo_d[:, :])

            def ctx_blk(et, jt):            # CTX [e-chunk et, j-tile jt]
                return CTXT[:, et * S + jt * 128: et * S + (jt + 1) * 128]

            # ---- GC[c, j] = sum_e G[c, e] ctx[e, j]  (Q/K path, once) ----
            # jh-outer: the four jh=0 groups need only the ctx first-halves,
            # which are the first DMAs to land.
            GC = [
                cpool.tile([128, S], F32R, name=f"gc{ct}", tag=f"gc{ct}")
                for ct in range(4)
            ]
            for jh in range(2):
                for ct in range(4):
                    gps = ps_s.tile([128, LI], F32, name="gps", tag="s")
                    for et in range(4):
                        nc.tensor.matmul(
                            gps[:],
                            GTS[:, et * C + ct * 128: et * C + (ct + 1) * 128],
                            CTXT[:, et * S + jh * LI: et * S + (jh + 1) * LI],
                            start=(et == 0),
                            stop=(et == 3),
                        )
                    nc.vector.tensor_copy(GC[ct][:, jh * LI:(jh + 1) * LI], gps[:])

            X = X0
            for ic in range(NCHUNK):
                isl = bass.ts(ic, LI)
                Xc = X
                if ic + 1 < NCHUNK:
                    X = load_x(ic + 1)      # prefetch next chunk
                # ST[j, i] = GC.T-contract X (+ q0[j]) ; PT = exp(ST).
                # The Z partial-sum tree (pairwise adds on the DVE) is
                # interleaved into the S-loop so each add issues as soon as
                # its exp operands exist; the final 128-partition fold is a
                # single ones-matmul placed after the first U-group so the
                # in-order PE stream never waits on the DVE.
                PT = []
                tpart = {}
                for jt in range(8):
                    sps = ps_s.tile([128, LI], F32, name="sps", tag="s")
                    for ct in range(4):
                        nc.tensor.matmul(
                            sps[:],
                            GC[ct][:, jt * 128:(jt + 1) * 128],
                            Xc[:, bass.ts(ct, LI)],
                            start=(ct == 0),
                            stop=(ct == 3),
                        )
                    p = ppool.tile([128, LI], F32R, name="pt", tag=f"p{jt}")
                    if has_q0:
                        nc.scalar.activation(
                            p[:], sps[:], EXP, bias=q0_s[:, jt:jt + 1]
                        )
                    else:
                        nc.scalar.activation(p[:], sps[:], EXP)
                    PT.append(p)
                    if jt in (1, 3, 5, 7):
                        t = zpool.tile([128, LI], F32, name="tp", tag=f"t{jt // 2}")
                        nc.vector.tensor_add(
                            t[:],
                            PT[jt - 1][:].bitcast(F32),
                            PT[jt][:].bitcast(F32),
                        )
                        tpart[jt // 2] = t
                    if jt == 3:
                        ta = zpool.tile([128, LI], F32, name="ta", tag="ta")
                        nc.vector.tensor_add(ta[:], tpart[0][:], tpart[1][:])
                    if jt == 7:
                        tb = zpool.tile([128, LI], F32, name="tb", tag="tb")
                        nc.vector.tensor_add(tb[:], tpart[2][:], tpart[3][:])
                        zt = zpool.tile([128, LI], F32R, name="zt", tag="zt")
                        nc.vector.tensor_add(zt[:], ta[:], tb[:])
                if ic == 0:
                    # VW[j, o] = sum_e ctx[e, j] WV[e, o] (V/out path, once).
                    # Emitted between chunk 0's S and U stages so it hides in
                    # the exp/Z latency instead of delaying the first chunk.
                    VW = []
                    for jt in range(8):
                        vps = ps_s.tile([128, E], F32, name="vps", tag="s")
                        for et in range(4):
                            nc.tensor.matmul(
                                vps[:],
                                ctx_blk(et, jt),
                                WVT[:, bass.ts(et, E)],
                                start=(et == 0),
                                stop=(et == 3),
                            )
                        vw = cpool.tile(
                            [128, E], F32R, name=f"vwt{jt}", tag=f"vwt{jt}"
                        )
                        nc.vector.tensor_copy(vw[:], vps[:])
                        VW.append(vw)
                # U[o, i] = VW.T-contract PT ; y = U * invZ (+ b_o)
                for ot in range(4):
                    ups = ps_u.tile([128, LI], F32, name="ups", tag="u")
                    for jt in range(8):
                        nc.tensor.matmul(
                            ups[:],
                            VW[jt][:, ot * 128:(ot + 1) * 128],
                            PT[jt][:],
                            start=(jt == 0),
                            stop=(jt == 7),
                        )
                    if ot == 0:
                        zps = ps_z.tile([1, LI], F32, name="zps", tag="z")
                        nc.tensor.matmul(
                            zps[:], ones_col[:], zt[:], start=True, stop=True
                        )
                        invz = zpool.tile([1, LI], F32, name="invz", tag="invz")
                        # full-precision reciprocal costs 3.3us on the DVE and
                        # gates the output chain; the fast approx (~18 correct
                        # bits, well beyond the fp32r noise floor) is ~5x
                        # faster. Z is strictly positive so the undefined
                        # edge cases (0/denorm/inf) cannot occur.
                        nc.vector.reciprocal_approx_fast(out=invz[:], in_=zps[:])
                        invz_rep = zpool.tile(
                            [128, LI], F32, name="invz_rep", tag="invzrep"
                        )
                        nc.gpsimd.partition_broadcast(invz_rep[:], invz[:])
                    o = opool.tile([128, LI], F32, name="ot", tag=f"o{ot}")
                    nc.vector.tensor_mul(o[:], ups[:], invz_rep[:])
                    if has_bo:
                        nc.vector.tensor_scalar_add(o[:], o[:], bo_s[:, ot:ot + 1])
                    nc.sync.dma_start(y_d[ot * 128:(ot + 1) * 128, isl], o[:])

    nc.compile()
    return nc


def kernel(**inputs) -> np.ndarray:
    global LAST_RESULTS
    x = np.asarray(inputs["x"], dtype=np.float32)
    context = np.asarray(inputs["context"], dtype=np.float32)
    W_pi = np.asarray(inputs["W_pi"], dtype=np.float64)
    b_pi = np.asarray(inputs["b_pi"], dtype=np.float64)
    W_q = np.asarray(inputs["W_q"], dtype=np.float64)
    b_q = np.asarray(inputs["b_q"], dtype=np.float64)
    W_k = np.asarray(inputs["W_k"], dtype=np.float64)
    W_v = np.asarray(inputs["W_v"], dtype=np.float64)
    b_v = np.asarray(inputs["b_v"], dtype=np.float64)
    W_po = np.asarray(inputs["W_po"], dtype=np.float64)
    b_po = np.asarray(inputs["b_po"], dtype=np.float64)

    scale = float(E) ** -0.5
    Wqpi = scale * (W_q @ W_pi)                            # [dq, c]
    G = (Wqpi.T @ W_k)                                     # [c, e]
    GT = _round_tf32(np.ascontiguousarray(G.T).astype(np.float32))  # [e, c]
    b_row = scale * (W_q @ b_pi + b_q)
    # per-KEY bias on the logits: q0[j] = (W_k.T b_row) . ctx[:, j] is handled
    # as an activation bias per j-partition, computed from ctx on the host
    # would be data-work; instead fold the e-space bias through the device GC
    # path is impossible (it multiplies ctx), so compute the per-j bias here
    # only when biases are actually nonzero (they are all zero in this
    # problem's inputs).
    q0_e = (W_k.T @ b_row).astype(np.float64)              # [e]
    WV = _round_tf32((W_po @ W_v).T.astype(np.float32))    # [e, o]
    b_o = (b_po + W_po @ b_v).astype(np.float32)           # [o]

    has_q0 = bool(np.any(q0_e))
    has_bo = bool(np.any(b_o))
    key = (has_q0, has_bo)
    if key not in _PROGRAM_CACHE:
        _PROGRAM_CACHE[key] = _build_program(has_q0, has_bo)
    nc = _PROGRAM_CACHE[key]

    ones_c = np.ones((128, 1), dtype=np.float32)
    in_maps = []
    for c in range(NCORES):
        ctx_mat = context[c].reshape(E, S)
        m = {
            "x": _round_tf32(x[c].reshape(C, L)),
            "ctx": _round_tf32(ctx_mat),
            "gt": GT,
            "wv": WV,
            "onec": ones_c,
        }
        if has_q0:
            # logits bias per key j: q0_e . ctx[:, j]  -> [S] -> [128, 8]
            q0j = (q0_e @ ctx_mat.astype(np.float64)).astype(np.float32)
            m["q0"] = np.ascontiguousarray(q0j.reshape(8, 128).T)
        if has_bo:
            m["bo"] = np.ascontiguousarray(b_o.reshape(4, 128).T)
        in_maps.append(m)

    res = run_bass_kernel_spmd(nc, in_maps, core_ids=list(range(NCORES)), trace=TRACE)
    LAST_RESULTS = res
    y = np.stack([res.results[c]["y"] for c in range(NCORES)], axis=0)
    return np.ascontiguousarray(y.reshape(NCORES, C, 64, 64).astype(np.float32))


# revision 15
# speedup vs baseline: 1.0242x; 1.0080x over previous
"""Cross-attention kernel for TRN2, data-parallel over batch (B=8) on 8 cores.

Reference computation per batch element:
    xt  = proj_in(x)              # [L=4096, E=512], 1x1 conv == matmul
    Q   = xt @ W_q.T + b_q
    K   = ctx @ W_k.T + b_k       # ctx: [S=1024, E]
    V   = ctx @ W_v.T + b_v
    att = softmax(Q @ K.T * scale)
    out = proj_out((att @ V).T)   # [C=512, 64, 64]

Host-side algebraic folds (exact up to fp rounding):
  * scale, W_pi, W_q, W_k fold into a single matrix on the Q/K path:
      G = (scale * W_q @ W_pi).T @ W_k ;  logits.T = (G @ ctx).T-contract X
    (the Q'.b_k rank-1 term is constant across keys -> softmax-invariant,
    dropped; a nonzero bias path reappears as per-partition q0 on GC^T X)
  * W_v and W_po fold:  WV = (W_po @ W_v).T ; b_o = b_po + W_po @ b_v
  * softmax normalization is applied at the very end (divide by Z), so the
    attention core is exp -> matmul -> scale-by-1/Z.

On-device, the data-dependent weight products are built ONCE per core
(GC = G.T-contract ctx, VW = ctx.T-contract WV), then every query chunk
needs only the two unavoidable attention GEMMs plus the Z column-sum:
  ST[j,i] = GC.T-contract X ; PT = exp(ST)
  Z[i]    = ones.T @ PT (column sums via PE)
  U[o,i]  = VW.T-contract PT
  y[o,i]  = U * (1/Z broadcast via GpSimd) + b_o

All matmuls run in fp32r (TF32-like: 11-bit mantissa, low 12 bits zero).
DRAM-sourced matmul operands are pre-rounded on the host; device-produced
operands (GC, VW, PT) are rounded by the producing engine via an fp32r
output dtype. PSUM accumulation stays full fp32.
"""

import numpy as np

import concourse.bass as bass
import concourse.mybir as mybir
import concourse.tile as tile
from concourse import bacc
from concourse.bass_utils import run_bass_kernel_spmd

F32 = mybir.dt.float32
F32R = mybir.dt.float32r
EXP = mybir.ActivationFunctionType.Exp

C = 512       # in channels
E = 512       # emb dim
L = 4096      # query length (64*64)
S = 1024      # key length (32*32)
LI = 512      # i-chunk (query) tile size
NCHUNK = L // LI
NCORES = 8

TRACE = False           # test harness can flip this before calling kernel()
LAST_RESULTS = None     # stashed BassKernelResults for the test harness

_PROGRAM_CACHE = {}


def _round_tf32(a: np.ndarray) -> np.ndarray:
    """Round fp32 mantissa to 11 explicit bits (round-to-nearest-even),
    zeroing the low 12 bits — the fp32r operand format."""
    a = np.ascontiguousarray(a, dtype=np.float32)
    b = a.view(np.uint32)
    r = (b + np.uint32(0x7FF) + ((b >> np.uint32(12)) & np.uint32(1))) & np.uint32(
        0xFFFFF000
    )
    return r.view(np.float32)


def _build_program(has_q0: bool, has_bo: bool):
    nc = bacc.Bacc(
        "TRN2",
        target_bir_lowering=False,
        debug=False,
        enable_asserts=False,
        num_devices=NCORES,
    )
    x_d = nc.dram_tensor("x", [C, L], F32R, kind="ExternalInput").ap()
    ctx_d = nc.dram_tensor("ctx", [E, S], F32R, kind="ExternalInput").ap()
    # gt arrives host-permuted into ct-major blocks: gt_d[p, ct*512+et*128+c']
    # = G.T[et*128+p, ct*128+c'], so the first GC group (ct=0) only needs the
    # first 256KB block and DMA runs stay 2KB-contiguous.
    gt_d = nc.dram_tensor("gt", [128, 4 * C], F32R, kind="ExternalInput").ap()
    wv_d = nc.dram_tensor("wv", [E, E], F32R, kind="ExternalInput").ap()
    onec_d = nc.dram_tensor("onec", [128, 1], F32R, kind="ExternalInput").ap()
    q0_d = bo_d = None
    if has_q0:
        q0_d = nc.dram_tensor("q0", [128, 8], F32, kind="ExternalInput").ap()
    if has_bo:
        bo_d = nc.dram_tensor("bo", [128, 4], F32, kind="ExternalInput").ap()
    y_d = nc.dram_tensor("y", [C, L], F32, kind="ExternalOutput").ap()

    def load_4stack(pool, dram_ap, width, name):
        """[4*128, width] DRAM -> [128, 4*width] SBUF tile (partition-chunk t
        lands at free offset t*width). One DMA per chunk so consumers of an
        individual chunk can start as soon as that chunk lands (subtile
        deps), and the four transfers spread across DMA queues."""
        t = pool.tile([128, 4 * width], F32R, name=name, tag=name)
        for tt in range(4):
            nc.sync.dma_start(
                t[:, tt * width:(tt + 1) * width],
                dram_ap[tt * 128:(tt + 1) * 128, :],
            )
        return t

    with tile.TileContext(nc) as tc:
        from contextlib import ExitStack

        with ExitStack() as ctx:
            cpool = ctx.enter_context(tc.tile_pool(name="consts", bufs=1))
            ps_s = ctx.enter_context(tc.tile_pool(name="ps_s", bufs=3, space="PSUM"))
            ps_z = ctx.enter_context(tc.tile_pool(name="ps_z", bufs=1, space="PSUM"))
            ps_u = ctx.enter_context(tc.tile_pool(name="ps_u", bufs=4, space="PSUM"))
            xpool = ctx.enter_context(tc.tile_pool(name="xp", bufs=2))
            ppool = ctx.enter_context(tc.tile_pool(name="pp", bufs=2))
            opool = ctx.enter_context(tc.tile_pool(name="op", bufs=2))
            zpool = ctx.enter_context(tc.tile_pool(name="zp", bufs=2))

            # ---- loads in latency-priority order --------------------------
            ones_col = cpool.tile([128, 1], F32R, name="ones_col")
            nc.sync.dma_start(ones_col[:], onec_d[:, :])
            # interleave gt chunks with ctx first-halves so the jh=0 GC
            # groups (which need gt[*] + ctx[*, :LI]) complete after ~2MB of
            # DMA instead of the full 3MB; ctx second-halves follow X0.
            GTS = cpool.tile([128, 4 * C], F32R, name="gstk", tag="gstk")
            CTXT = cpool.tile([128, 4 * S], F32R, name="cstk", tag="cstk")
            nc.sync.dma_start(GTS[:, 0:512], gt_d[:, 0:512])
            for tt in range(4):
                nc.sync.dma_start(
                    CTXT[:, tt * S:tt * S + LI],
                    ctx_d[tt * 128:(tt + 1) * 128, 0:LI],
                )
            for ctb in range(1, 4):
                nc.sync.dma_start(
                    GTS[:, ctb * 512:(ctb + 1) * 512], gt_d[:, ctb * 512:(ctb + 1) * 512]
                )

            def load_x(ic):
                xt = xpool.tile([128, 4 * LI], F32R, name="xc", tag="x")
                nc.sync.dma_start(
                    xt[:].rearrange("p (t c) -> p t c", c=LI),
                    x_d[:, bass.ts(ic, LI)].rearrange("(t p) c -> p t c", p=128),
                )
                return xt

            X0 = load_x(0)                                        # prefetch chunk 0
            for tt in range(4):
                nc.sync.dma_start(
                    CTXT[:, tt * S + LI:(tt + 1) * S],
                    ctx_d[tt * 128:(tt + 1) * 128, LI:S],
                )
            WVT = load_4stack(cpool, wv_d[:, :], E, "wstk")       # [128, 4*E]
            q0_s = bo_s = None
            if has_q0:
                q0_s = cpool.tile([128, 8], F32, name="q0s")
                nc.sync.dma_start(q0_s[:], q0_d[:, :])
            if has_bo:
                bo_s = cpool.tile([128, 4], F32, name="bos")
                nc.sync.dma_start(bo_s[:], bo_d[:, :])

            def ctx_blk(et, jt):            # CTX [e-chunk et, j-tile jt]
                return CTXT[:, et * S + jt * 128: et * S + (jt + 1) * 128]

            # ---- GC[c, j] = sum_e G[c, e] ctx[e, j]  (Q/K path, once) ----
            # jh-outer: the four jh=0 groups need only the ctx first-halves,
            # which are the first DMAs to land.
            GC = [
                cpool.tile([128, S], F32R, name=f"gc{ct}", tag=f"gc{ct}")
                for ct in range(4)
            ]
            for jh in range(2):
                for ct in range(4):
                    gps = ps_s.tile([128, LI], F32, name="gps", tag="s")
                    for et in range(4):
                        nc.tensor.matmul(
                            gps[:],
                            GTS[:, ct * 512 + et * 128: ct * 512 + (et + 1) * 128],
                            CTXT[:, et * S + jh * LI: et * S + (jh + 1) * LI],
                            start=(et == 0),
                            stop=(et == 3),
                        )
                    nc.vector.tensor_copy(GC[ct][:, jh * LI:(jh + 1) * LI], gps[:])

            X = X0
            for ic in range(NCHUNK):
                isl = bass.ts(ic, LI)
                Xc = X
                if ic + 1 < NCHUNK:
                    X = load_x(ic + 1)      # prefetch next chunk
                # ST[j, i] = GC.T-contract X (+ q0[j]) ; PT = exp(ST).
                # The Z partial-sum tree (pairwise adds on the DVE) is
                # interleaved into the S-loop so each add issues as soon as
                # its exp operands exist; the final 128-partition fold is a
                # single ones-matmul placed after the first U-group so the
                # in-order PE stream never waits on the DVE.
                PT = []
                tpart = {}
                for jt in range(8):
                    sps = ps_s.tile([128, LI], F32, name="sps", tag="s")
                    for ct in range(4):
                        nc.tensor.matmul(
                            sps[:],
                            GC[ct][:, jt * 128:(jt + 1) * 128],
                            Xc[:, bass.ts(ct, LI)],
                            start=(ct == 0),
                            stop=(ct == 3),
                        )
                    p = ppool.tile([128, LI], F32R, name="pt", tag=f"p{jt}")
                    if has_q0:
                        nc.scalar.activation(
                            p[:], sps[:], EXP, bias=q0_s[:, jt:jt + 1]
                        )
                    else:
                        nc.scalar.activation(p[:], sps[:], EXP)
                    PT.append(p)
                    if jt in (1, 3, 5, 7):
                        t = zpool.tile([128, LI], F32, name="tp", tag=f"t{jt // 2}")
                        nc.vector.tensor_add(
                            t[:],
                            PT[jt - 1][:].bitcast(F32),
                            PT[jt][:].bitcast(F32),
                        )
                        tpart[jt // 2] = t
                    if jt == 3:
                        ta = zpool.tile([128, LI], F32, name="ta", tag="ta")
                        nc.vector.tensor_add(ta[:], tpart[0][:], tpart[1][:])
                    if jt == 7:
                        tb = zpool.tile([128, LI], F32, name="tb", tag="tb")
                        nc.vector.tensor_add(tb[:], tpart[2][:], tpart[3][:])
                        zt = zpool.tile([128, LI], F32R, name="zt", tag="zt")
                        nc.vector.tensor_add(zt[:], ta[:], tb[:])
                if ic == 0:
                    # VW[j, o] = sum_e ctx[e, j] WV[e, o] (V/out path, once).
                    # Emitted between chunk 0's S and U stages so it hides in
                    # the exp/Z latency instead of delaying the first chunk.
                    VW = []
                    for jt in range(8):
                        vps = ps_s.tile([128, E], F32, name="vps", tag="s")
                        for et in range(4):
                            nc.tensor.matmul(
                                vps[:],
                                ctx_blk(et, jt),
                                WVT[:, bass.ts(et, E)],
                                start=(et == 0),
                                stop=(et == 3),
                            )
                        vw = cpool.tile(
                            [128, E], F32R, name=f"vwt{jt}", tag=f"vwt{jt}"
                        )
                        nc.vector.tensor_copy(vw[:], vps[:])
                        VW.append(vw)
                # U[o, i] = VW.T-contract PT ; y = U * invZ (+ b_o)
                for ot in range(4):
                    ups = ps_u.tile([128, LI], F32, name="ups", tag="u")
                    for jt in range(8):
                        nc.tensor.matmul(
                            ups[:],
                            VW[jt][:, ot * 128:(ot + 1) * 128],
                            PT[jt][:],
                            start=(jt == 0),
                            stop=(jt == 7),
                        )
                    if ot == 0:
                        zps = ps_z.tile([1, LI], F32, name="zps", tag="z")
                        nc.tensor.matmul(
                            zps[:], ones_col[:], zt[:], start=True, stop=True
                        )
                        invz = zpool.tile([1, LI], F32, name="invz", tag="invz")
                        # full-precision reciprocal costs 3.3us on the DVE and
                        # gates the output chain; the fast approx (~18 correct
                        # bits, well beyond the fp32r noise floor) is ~5x
                        # faster. Z is strictly positive so the undefined
                        # edge cases (0/denorm/inf) cannot occur.
                        nc.vector.reciprocal_approx_fast(out=invz[:], in_=zps[:])
                        invz_rep = zpool.tile(
                            [128, LI], F32, name="invz_rep", tag="invzrep"
                        )
                        nc.gpsimd.partition_broadcast(invz_rep[:], invz[:])
                    o = opool.tile([128, LI], F32, name="ot", tag=f"o{ot}")
                    nc.vector.tensor_mul(o[:], ups[:], invz_rep[:])
                    if has_bo:
                        nc.vector.tensor_scalar_add(o[:], o[:], bo_s[:, ot:ot + 1])
                    nc.sync.dma_start(y_d[ot * 128:(ot + 1) * 128, isl], o[:])

    nc.compile()
    return nc


def kernel(**inputs) -> np.ndarray:
    global LAST_RESULTS
    x = np.asarray(inputs["x"], dtype=np.float32)
    context = np.asarray(inputs["context"], dtype=np.float32)
    W_pi = np.asarray(inputs["W_pi"], dtype=np.float64)
    b_pi = np.asarray(inputs["b_pi"], dtype=np.float64)
    W_q = np.asarray(inputs["W_q"], dtype=np.float64)
    b_q = np.asarray(inputs["b_q"], dtype=np.float64)
    W_k = np.asarray(inputs["W_k"], dtype=np.float64)
    W_v = np.asarray(inputs["W_v"], dtype=np.float64)
    b_v = np.asarray(inputs["b_v"], dtype=np.float64)
    W_po = np.asarray(inputs["W_po"], dtype=np.float64)
    b_po = np.asarray(inputs["b_po"], dtype=np.float64)

    scale = float(E) ** -0.5
    Wqpi = scale * (W_q @ W_pi)                            # [dq, c]
    G = (Wqpi.T @ W_k)                                     # [c, e]
    GT = _round_tf32(np.ascontiguousarray(G.T).astype(np.float32))  # [e, c]
    # ct-major block permutation: A[p, ct*512+et*128+c'] = GT[et*128+p, ct*128+c']
    GT = np.ascontiguousarray(
        GT.reshape(4, 128, 4, 128).transpose(1, 2, 0, 3).reshape(128, 4 * C)
    )
    b_row = scale * (W_q @ b_pi + b_q)
    # per-KEY bias on the logits: q0[j] = (W_k.T b_row) . ctx[:, j] is handled
    # as an activation bias per j-partition, computed from ctx on the host
    # would be data-work; instead fold the e-space bias through the device GC
    # path is impossible (it multiplies ctx), so compute the per-j bias here
    # only when biases are actually nonzero (they are all zero in this
    # problem's inputs).
    q0_e = (W_k.T @ b_row).astype(np.float64)              # [e]
    WV = _round_tf32((W_po @ W_v).T.astype(np.float32))    # [e, o]
    b_o = (b_po + W_po @ b_v).astype(np.float32)           # [o]

    has_q0 = bool(np.any(q0_e))
    has_bo = bool(np.any(b_o))
    key = (has_q0, has_bo)
    if key not in _PROGRAM_CACHE:
        _PROGRAM_CACHE[key] = _build_program(has_q0, has_bo)
    nc = _PROGRAM_CACHE[key]

    ones_c = np.ones((128, 1), dtype=np.float32)
    in_maps = []
    for c in range(NCORES):
        ctx_mat = context[c].reshape(E, S)
        m = {
            "x": _round_tf32(x[c].reshape(C, L)),
            "ctx": _round_tf32(ctx_mat),
            "gt": GT,
            "wv": WV,
            "onec": ones_c,
        }
        if has_q0:
            # logits bias per key j: q0_e . ctx[:, j]  -> [S] -> [128, 8]
            q0j = (q0_e @ ctx_mat.astype(np.float64)).astype(np.float32)
            m["q0"] = np.ascontiguousarray(q0j.reshape(8, 128).T)
        if has_bo:
            m["bo"] = np.ascontiguousarray(b_o.reshape(4, 128).T)
        in_maps.append(m)

    res = run_bass_kernel_spmd(nc, in_maps, core_ids=list(range(NCORES)), trace=TRACE)
    LAST_RESULTS = res
    y = np.stack([res.results[c]["y"] for c in range(NCORES)], axis=0)
    return np.ascontiguousarray(y.reshape(NCORES, C, 64, 64).astype(np.float32))
